# revision 31
# baseline (speedup 1.0000x reference)
"""CosFace (LMCL) loss + center loss, sharded over 8 Trainium2 NeuronCores.

Strategy (classification parallel over the class dim, per sharding hint):
  - weight [50000,128] is zero-padded to [50176,128] and split into 8 shards
    of 6272 rows (49 tiles of 128).
  - Each core: normalizes its weight shard + the (replicated) feature matrix,
    computes cos = f_norm @ w_norm.T for its classes, and returns per-sample
    partial sums A[n] = sum_c exp(s*cos[n,c] - 30).  The feature-norm scale is
    folded into the activation's per-partition scale, so the matmul consumes
    raw transposed features.  exp+sum is fused in one ScalarE pass (accum_out).
  - Every core also computes (redundantly, it is tiny) the per-sample target
    cosine t[n] = f_norm[n].w_norm[label[n]] and center distances
    q[n] = ||f[n]-w[label[n]]||^2 from a host-gathered wl = weight[label].
  - Host combines in float64: subtracts the exact padding contribution
    (pad rows are exactly zero -> each contributes exp(-30)), applies the
    CosFace margin correction for the target class, takes log, and assembles
    loss = mean(lse - s*(t-m)) + lambda*0.5*sum(q).
"""

import numpy as np

import concourse.bass as bass
import concourse.mybir as mybir
import concourse.tile as tile
from concourse.bass_utils import run_bass_kernel_spmd
from concourse.masks import make_identity

# ---------------------------------------------------------------------------
# Workaround for this container's walrus build: instructions carrying more
# than one semaphore wait fail codegen ("Too many sync wait commands" /
# setupSyncWait internal errors).  Tile attaches multiple waits to one
# instruction when it depends on several producers.  Post-pass: move all but
# one wait onto standalone single-wait EventSemaphore instructions inserted
# immediately before, on the same engine.
# ---------------------------------------------------------------------------


def _split_multi_waits(nc):
    for fn in nc.m.functions:
        for bb in fn.blocks:
            insts = bb.instructions
            out = []
            changed = False
            for inst in insts:
                si = inst.sync_info
                if si is not None and len(si.on_wait) > 1:
                    waits = list(si.on_wait)
                    for w in waits[:-1]:
                        ev = mybir.InstEventSemaphore(
                            name=nc.get_next_instruction_name(), ins=[], outs=[]
                        )
                        ev.engine = inst.engine
                        ev.sync_info = mybir.SyncInfo(on_wait=[w], on_update=[])
                        nc.register_instruction(ev, overwrite=True)
                        out.append(ev)
                    si.on_wait[:] = waits[-1:]
                    changed = True
                out.append(inst)
            if changed:
                bb.instructions = out

# ---------------------------------------------------------------------------

F32 = mybir.dt.float32
F32R = mybir.dt.float32r
AF = mybir.ActivationFunctionType
AX = mybir.AxisListType

N_CORES = 8
N = 1024
C = 50000
D = 128
P = 128
NCH = N // P  # 8 sample chunks
CT = 49  # class tiles per core
CLOC = CT * P  # 6272 local classes
CPAD = N_CORES * CLOC  # 50176
NPAD = CPAD - C  # 176 zero rows (all on the last core)

S_SCALE = 30.0
M_MARGIN = 0.35
LAMBDA = 0.01
EXP_BIAS = -30.0  # exp(s*cos + EXP_BIAS); s*cos <= 30 so sums stay in fp32
EPS2 = 1e-16  # matches torch CosineSimilarity eps=1e-8 on squared norms

# class-band structure: prep groups and main-loop bands are aligned.  Small
# leading bands let the exp pipeline start while later groups still stream.
BAND_TILES = [4, 8, 12, 12, 12, 1]  # 49 tiles of 128 classes
assert sum(BAND_TILES) == 49
BANDS = []
_t0 = 0
for _bt in BAND_TILES:
    BANDS.append((_t0, _t0 + _bt))
    _t0 += _bt
NB = len(BANDS)
MMB = 512  # matmul moving-block width


def _build_program(loop_iters=None):
    nc = bass.Bass(
        "TRN2", target_bir_lowering=False, debug=False, num_devices=N_CORES
    )
    w = nc.dram_tensor("w", [CLOC, D], F32, kind="ExternalInput").ap()
    f = nc.dram_tensor("f", [N, D], F32, kind="ExternalInput").ap()
    wl = nc.dram_tensor("wl", [N, D], F32, kind="ExternalInput").ap()
    o = nc.dram_tensor("o", [P, 3 * NCH], F32, kind="ExternalOutput").ap()

    with tile.TileContext(nc) as tc:
        from contextlib import ExitStack

        with ExitStack() as ctx:
            cpool = ctx.enter_context(tc.tile_pool(name="const", bufs=1))
            big = ctx.enter_context(tc.tile_pool(name="big", bufs=1))
            scr = ctx.enter_context(tc.tile_pool(name="scr", bufs=1))
            st = ctx.enter_context(tc.tile_pool(name="stats", bufs=1))
            psum = ctx.enter_context(
                tc.tile_pool(name="psum", bufs=2, space="PSUM")
            )
            psum_p = ctx.enter_context(
                tc.tile_pool(name="psum_p", bufs=2, space="PSUM")
            )

            ident = cpool.tile([P, P], F32, tag="ident")
            make_identity(nc, ident[:])
            ebias = cpool.tile([P, 1], F32, tag="ebias")
            nc.gpsimd.memset(ebias[:], EXP_BIAS)

            w_nat = big.tile([P, CLOC], F32, tag="w_nat")
            wsc = big.tile([P, CLOC], F32, tag="wsc")  # squares, then scaled w
            wT = big.tile([P, CLOC], F32R, tag="wT")
            f_nat = big.tile([P, N], F32, tag="f_nat")
            wl_nat = big.tile([P, N], F32, tag="wl_nat")
            fT = big.tile([P, N], F32R, tag="fT")

            sq_f = scr.tile([P, N], F32, tag="sq_f")
            sq_wl = scr.tile([P, N], F32, tag="sq_wl")
            fw = scr.tile([P, N], F32, tag="fw")
            dif = scr.tile([P, N], F32, tag="dif")
            dsq = scr.tile([P, N], F32, tag="dsq")

            wns = st.tile([P, CT], F32, tag="wns")
            wln = st.tile([P, CT], F32, tag="wln")
            rw = st.tile([P, CT], F32, tag="rw")
            fns = st.tile([P, NCH], F32, tag="fns")
            fln = st.tile([P, NCH], F32, tag="fln")
            rf = st.tile([P, NCH], F32, tag="rf")
            wlns = st.tile([P, NCH], F32, tag="wlns")
            wlln = st.tile([P, NCH], F32, tag="wlln")
            rwl = st.tile([P, NCH], F32, tag="rwl")
            dots = st.tile([P, NCH], F32, tag="dots")
            tmp8 = st.tile([P, NCH], F32, tag="tmp8")
            acc = st.tile([P, NB * NCH], F32, tag="acc")
            outt = st.tile([P, 3 * NCH], F32, tag="outt")

            w3d_dram = w.rearrange("(t p) d -> p t d", p=P)
            w_nat3 = w_nat[:].rearrange("p (t d) -> p t d", d=D)
            wsc3 = wsc[:].rearrange("p (t d) -> p t d", d=D)
            sq3 = lambda ap: ap.rearrange("p (c d) -> p c d", d=D)

            # ---- warm the ACT table set (Ln/Exp) off the critical path ------
            dummy = cpool.tile([P, 1], F32, tag="dummy")
            nc.scalar.activation(dummy[:], ebias[:], AF.Exp)

            from contextlib import nullcontext

            loop_cm = (
                tc.For_i(
                    0,
                    loop_iters,
                    1,
                    hint_engines=(
                        mybir.EngineType.PE,
                        mybir.EngineType.Activation,
                        mybir.EngineType.DVE,
                    ),
                )
                if loop_iters is not None
                else nullcontext()
            )
            # body below is indented under this context
            _body_cm = loop_cm.__enter__()

            # ---- input DMAs (one HWDGE ring, issue order = priority) --------
            f3d_dram = f.rearrange("(c p) d -> p c d", p=P)
            f_nat3 = f_nat[:].rearrange("p (c d) -> p c d", d=D)
            wl3d_dram = wl.rearrange("(c p) d -> p c d", p=P)
            wl_nat3 = wl_nat[:].rearrange("p (c d) -> p c d", d=D)
            nc.sync.dma_start(f_nat3[:], f3d_dram[:])
            nc.sync.dma_start(
                w_nat3[:, BANDS[0][0] : BANDS[0][1], :],
                w3d_dram[:, BANDS[0][0] : BANDS[0][1], :],
            )
            nc.sync.dma_start(
                w_nat3[:, BANDS[1][0] : BANDS[1][1], :],
                w3d_dram[:, BANDS[1][0] : BANDS[1][1], :],
            )
            for g0, g1 in BANDS[2:4]:
                nc.sync.dma_start(w_nat3[:, g0:g1, :], w3d_dram[:, g0:g1, :])
            nc.sync.dma_start(wl_nat3[:], wl3d_dram[:])
            for g0, g1 in BANDS[4:]:
                nc.sync.dma_start(w_nat3[:, g0:g1, :], w3d_dram[:, g0:g1, :])

            # ---- streamed weight prep: per 16-tile group --------------------
            # square -> rowsum -> 1/norm (exp(-0.5*ln(max(.,eps^2))))
            # -> scale rows (free-dim broadcast) -> PE transpose -> wT
            def w_prep_group(g0, g1, scale_eng=None):
                lo, hi = g0 * D, g1 * D
                nc.vector.tensor_mul(
                    wsc[:, lo:hi], w_nat[:, lo:hi], w_nat[:, lo:hi]
                )
                nc.vector.reduce_sum(
                    wns[:, g0:g1], wsc3[:, g0:g1, :], axis=AX.X
                )
                nc.vector.tensor_scalar_max(
                    wns[:, g0:g1], wns[:, g0:g1], EPS2
                )
                nc.scalar.activation(wln[:, g0:g1], wns[:, g0:g1], AF.Ln)
                nc.scalar.activation(
                    rw[:, g0:g1], wln[:, g0:g1], AF.Exp, scale=-0.5
                )
                rw_b = (
                    rw[:, g0:g1]
                    .unsqueeze(2)
                    .broadcast_to((P, g1 - g0, D))
                )
                (scale_eng or nc.vector).tensor_mul(
                    wsc3[:, g0:g1, :], w_nat3[:, g0:g1, :], rw_b
                )
                for s0 in range(g0, g1, 4):
                    s1 = min(s0 + 4, g1)
                    pt = psum_p.tile([P, MMB], F32, tag="pp")
                    for j in range(s1 - s0):
                        nc.tensor.transpose(
                            pt[:, j * P : (j + 1) * P],
                            wsc3[:, s0 + j, :],
                            ident[:],
                        )
                    nc.vector.tensor_copy(
                        wT[:, s0 * P : s1 * P], pt[:, : (s1 - s0) * P]
                    )

            w_prep_group(*BANDS[0])

            # ---- feature stats (DVE squares/reduce, ACT norms) --------------
            nc.vector.tensor_mul(sq_f[:], f_nat[:], f_nat[:])
            nc.vector.reduce_sum(fns[:], sq3(sq_f[:]), axis=AX.X)
            nc.gpsimd.tensor_scalar_max(fns[:], fns[:], EPS2)
            nc.scalar.activation(fln[:], fns[:], AF.Ln)
            nc.scalar.activation(rf[:], fln[:], AF.Exp, scale=-0.5)
            # fold the feature 1/norm into fT so the hot exp ops use an
            # immediate scale (per-partition scale APs stay off the fast path)
            rf_b = rf[:].unsqueeze(2).broadcast_to((P, NCH, D))
            fn3 = sq_f[:].rearrange("p (c d) -> p c d", d=D)  # reuse scratch
            nc.vector.tensor_mul(fn3, f_nat3[:], rf_b)

            # ---- transpose normalized features into fT [d, n_local] (PE) ----
            for h in range(2):
                ptf = psum_p.tile([P, MMB], F32, tag="pp")
                for j in range(4):
                    ch = 4 * h + j
                    nc.tensor.transpose(
                        ptf[:, j * P : (j + 1) * P], fn3[:, ch, :], ident[:]
                    )
                nc.vector.tensor_copy(
                    fT[:, h * MMB : (h + 1) * MMB], ptf[:]
                )

            w_prep_group(*BANDS[1])

            # ---- label-weight stats (for t) ---------------------------------
            nc.gpsimd.tensor_mul(sq_wl[:], wl_nat[:], wl_nat[:])
            nc.vector.reduce_sum(wlns[:], sq3(sq_wl[:]), axis=AX.X)
            nc.gpsimd.tensor_scalar_max(wlns[:], wlns[:], EPS2)
            nc.scalar.activation(wlln[:], wlns[:], AF.Ln)
            nc.scalar.activation(rwl[:], wlln[:], AF.Exp, scale=-0.5)

            for gb in BANDS[2:]:
                w_prep_group(*gb)

            # ---- main loop: matmul + fused exp/row-sum (ScalarE) ------------
            # band-outer so each band only needs its own prep group done
            for bi, (t0, t1) in enumerate(BANDS):
                off, gw = t0 * P, (t1 - t0) * P
                for ch in range(NCH):
                    lhs = fT[:, ch * P : (ch + 1) * P]
                    pt = psum.tile([P, 1536], F32, tag="ps")
                    for b in range(0, gw, MMB):
                        bw = min(MMB, gw - b)
                        nc.tensor.matmul(
                            pt[:, b : b + bw],
                            lhs,
                            wT[:, off + b : off + b + bw],
                            start=True,
                            stop=True,
                        )
                    nc.scalar.activation(
                        pt[:, :gw],
                        pt[:, :gw],
                        AF.Exp,
                        bias=ebias[:],
                        scale=S_SCALE,
                        accum_out=acc[:, NB * ch + bi : NB * ch + bi + 1],
                    )
                    if bi == NB - 1:
                        nc.vector.reduce_sum(
                            outt[:, ch : ch + 1],
                            acc[:, NB * ch : NB * ch + NB],
                            axis=AX.X,
                        )

            # ---- target cosine t and center distances q (fills gaps) --------
            nc.gpsimd.tensor_mul(fw[:], f_nat[:], wl_nat[:])
            nc.vector.reduce_sum(dots[:], sq3(fw[:]), axis=AX.X)
            nc.gpsimd.tensor_sub(dif[:], f_nat[:], wl_nat[:])
            nc.gpsimd.tensor_mul(dsq[:], dif[:], dif[:])
            nc.vector.reduce_sum(
                outt[:, 2 * NCH : 3 * NCH], sq3(dsq[:]), axis=AX.X
            )
            nc.gpsimd.tensor_mul(tmp8[:], dots[:], rf[:])
            nc.gpsimd.tensor_mul(outt[:, NCH : 2 * NCH], tmp8[:], rwl[:])

            nc.sync.dma_start(o[:], outt[:])

            loop_cm.__exit__(None, None, None)

    _split_multi_waits(nc)
    return nc


_NC_CACHE = None


def _get_program():
    global _NC_CACHE
    if _NC_CACHE is None:
        _NC_CACHE = _build_program()
    return _NC_CACHE


def _build_program_loop(iters):
    return _build_program(loop_iters=iters)


def _host_prepare(feature, weight, label):
    feature = np.ascontiguousarray(np.asarray(feature, dtype=np.float32))
    weight = np.asarray(weight, dtype=np.float32)
    label = np.asarray(label).astype(np.int64)
    wl = np.ascontiguousarray(weight[label])
    w_pad = np.zeros((CPAD, D), dtype=np.float32)
    w_pad[:C] = weight
    in_maps = []
    for k in range(N_CORES):
        shard = np.ascontiguousarray(w_pad[k * CLOC : (k + 1) * CLOC])
        in_maps.append({"w": shard, "f": feature, "wl": wl})
    return in_maps


def _host_combine(results):
    # device layout: out[p, col]; sample n = ch*128 + p
    outs = [np.asarray(r["o"], dtype=np.float64) for r in results]
    A = sum(o[:, 0:NCH] for o in outs)  # [128, 8]
    A_n = A.T.reshape(N)  # [1024]
    t_n = outs[0][:, NCH : 2 * NCH].T.reshape(N)
    q_n = outs[0][:, 2 * NCH : 3 * NCH].T.reshape(N)

    S_raw = A_n - NPAD * np.exp(EXP_BIAS)
    S_fix = (
        S_raw
        - np.exp(S_SCALE * t_n + EXP_BIAS)
        + np.exp(S_SCALE * (t_n - M_MARGIN) + EXP_BIAS)
    )
    lse = np.log(S_fix) - EXP_BIAS
    target_logit = S_SCALE * (t_n - M_MARGIN)
    loss_lmc = np.mean(lse - target_logit)
    loss_c = 0.5 * np.sum(q_n)
    return np.float32(loss_lmc + LAMBDA * loss_c)


def kernel(feature, weight, label):
    nc = _get_program()
    in_maps = _host_prepare(feature, weight, label)
    res = run_bass_kernel_spmd(nc, in_maps, list(range(N_CORES)))
    return _host_combine(res.results)


def run_sim(feature, weight, label, core=7):
    """Simulate a single core (default: the padded one) and return its raw
    output tile plus the in_maps used — for numeric validation offline."""
    from concourse.bass_interp import MultiCoreSim

    nc = _get_program()
    in_maps = _host_prepare(feature, weight, label)
    sim = MultiCoreSim(nc, 1)
    for name, arr in in_maps[core].items():
        sim.cores[0].tensor(name)[:] = arr
    sim.simulate()
    return np.array(sim.cores[0].tensor("o")), in_maps
